# revision 12
# baseline (speedup 1.0000x reference)
"""GRU-D style GRUI encoder kernel for Trainium2 (Bass/Tile), 8 NeuronCores.

Data-parallel over batch B=256 across 8 cores (32 seqs/core), with the 32
sequences split into two groups A/B of 16 that are software-pipelined at
sub-step granularity: while group A's recurrence chain is in its ACT/DVE
phase, group B's matmuls run on the PE, halving the effective per-step
dependency-chain latency.

Other structure (informed by trace analysis):
  - beta = exp(-relu(delta@Wtd+b)) for ALL T steps is computed upfront
    (no ACT-table thrash between Exp and Sigmoid in the steady loop).
  - per step each group has ONE psum region [128, 6, 16] = [r0 r1 m0 m1
    h0 h1], seeded by a single identity-inject matmul.
  - r|mu sigmoid merged into one ACT op per group (ACT op count is the
    per-step budget limit); tanh separate.
  - state update hb' = p + w*(hhat-hb) with p = beta*hb computed on
    GPSIMD right at step start and w = beta*mu after mu — both off the
    critical path; the tail after tanh is 3 short DVE ops per group.
  - gate-x GEMMs for chunk c+1 are spread one-per-~2.5-steps across
    chunk c's emission so the PE never sees a burst at chunk boundaries.
"""

import numpy as np
import ml_dtypes
from contextlib import ExitStack

import concourse.bass as bass
import concourse.bacc as bacc
import concourse.tile as tile
from concourse import mybir
from concourse.bass_utils import run_bass_kernel_spmd
from concourse.masks import make_identity

B, T, D, H = 256, 512, 128, 256
NCORES = 8
BL = B // NCORES          # 32 sequences per core
GB = 16                   # sequences per pipeline group (2 groups)
C = 64                    # steps per chunk
NCHUNK = T // C
QSTEPS = 16               # steps per precompute GEMM group (N = 16*32 = 512)

FP32 = mybir.dt.float32
BF16 = mybir.dt.bfloat16
AF = mybir.ActivationFunctionType
ALU = mybir.AluOpType

_cache = {}


def _build():
    nc = bacc.Bacc("TRN2", target_bir_lowering=False, debug=False,
                   num_devices=NCORES)

    xT = nc.dram_tensor("xT", [D, T * BL], BF16, kind="ExternalInput")
    dTs = nc.dram_tensor("dTs", [D, T * BL], BF16, kind="ExternalInput")
    wx_rmu_d = nc.dram_tensor("wx_rmu", [D, 2 * H], BF16, kind="ExternalInput")
    wx_h_d = nc.dram_tensor("wx_h", [D, H], BF16, kind="ExternalInput")
    wtd_d = nc.dram_tensor("wtd", [D, H], BF16, kind="ExternalInput")
    wh_all_d = nc.dram_tensor("wh_all", [128, 12 * 128], BF16,
                              kind="ExternalInput")
    b_rmu_d = nc.dram_tensor("b_rmu", [128, 4], FP32, kind="ExternalInput")
    b_h_d = nc.dram_tensor("b_h", [128, 2], FP32, kind="ExternalInput")
    nb_td_d = nc.dram_tensor("nb_td", [128, 2], FP32, kind="ExternalInput")
    out_d = nc.dram_tensor("hT_out", [128, 2 * BL], FP32, kind="ExternalOutput")

    with ExitStack() as ctx:
        tc = ctx.enter_context(tile.TileContext(nc))
        wpool = ctx.enter_context(tc.tile_pool(name="weights", bufs=1))
        betapool = ctx.enter_context(tc.tile_pool(name="beta", bufs=1))
        xpool = ctx.enter_context(tc.tile_pool(name="xin", bufs=2))
        gxpool = ctx.enter_context(tc.tile_pool(name="gx", bufs=2))
        pre_ps = ctx.enter_context(tc.tile_pool(name="pre_ps", bufs=2,
                                                space="PSUM"))
        sps_pool = ctx.enter_context(tc.tile_pool(name="sps", bufs=2,
                                                  space="PSUM"))
        spool = ctx.enter_context(tc.tile_pool(name="state", bufs=3))

        # --- weights / constants into SBUF ---
        wh_all = wpool.tile([128, 12, 128], BF16)
        nc.sync.dma_start(wh_all.rearrange("p a b -> p (a b)"), wh_all_d[:, :])
        wx_rmu = wpool.tile([128, 2 * H], BF16)
        nc.sync.dma_start(wx_rmu, wx_rmu_d[:, :])
        wx_h = wpool.tile([128, H], BF16)
        nc.sync.dma_start(wx_h, wx_h_d[:, :])
        wtd = wpool.tile([128, H], BF16)
        nc.sync.dma_start(wtd, wtd_d[:, :])
        b_rmu = wpool.tile([128, 4], FP32)
        nc.sync.dma_start(b_rmu, b_rmu_d[:, :])
        b_h = wpool.tile([128, 2], FP32)
        nc.sync.dma_start(b_h, b_h_d[:, :])
        nb_td = wpool.tile([128, 2], FP32)
        nc.sync.dma_start(nb_td, nb_td_d[:, :])
        ident = wpool.tile([128, 128], BF16)
        make_identity(nc, ident)

        # Touch bias tiles from DVE once so later TSP/ACT consumers don't
        # carry the DMA wait (walrus rejects TSP with 2 sync waits).
        scratch = wpool.tile([128, 4], FP32, tag="scratch")
        nc.vector.tensor_copy(scratch, b_rmu)
        scratch2 = wpool.tile([128, 2], FP32, tag="scratch2")
        nc.vector.tensor_copy(scratch2, b_h)
        scratch3 = wpool.tile([128, 2], FP32, tag="scratch3")
        nc.vector.tensor_copy(scratch3, nb_td)

        # beta for every step: [p, t, gb, k, b]
        bet_all = betapool.tile([128, T, 2, 2, GB], BF16)

        # ---------- Phase 1: all temporal-decay betas upfront ----------
        for c in range(NCHUNK):
            dch = xpool.tile([128, C * BL], BF16, tag="dch")
            nc.sync.dma_start(dch, dTs[:, c * C * BL:(c + 1) * C * BL])
            for q in range(C // QSTEPS):
                nsl = slice(q * QSTEPS * BL, (q + 1) * QSTEPS * BL)
                for k in range(2):
                    bps = pre_ps.tile([128, QSTEPS * BL], FP32, tag="bps")
                    nc.tensor.matmul(bps, wtd[:, k * 128:(k + 1) * 128],
                                     dch[:, nsl], start=True, stop=True)
                    # exp(-(z + b)) ; clamped to <=1 below (== exp(-relu))
                    nc.scalar.activation(
                        bet_all[:, c * C + q * QSTEPS:
                                c * C + (q + 1) * QSTEPS, :, k, :],
                        bps.rearrange("p (t g b) -> p t g b", g=2, b=GB),
                        AF.Exp, bias=nb_td[:, k:k + 1], scale=-1.0)
            nc.vector.tensor_scalar_min(
                bet_all[:, c * C:(c + 1) * C].rearrange(
                    "p t g k b -> p (t g k b)"),
                bet_all[:, c * C:(c + 1) * C].rearrange(
                    "p t g k b -> p (t g k b)"), 1.0)

        # ---------- Phase 2: gate-x precompute + recurrence ----------
        def emit_pre_unit(c, u):
            """One x-GEMM + evacuation for chunk c, unit u (0..23)."""
            q, m = divmod(u, 6)
            nsl = slice(q * QSTEPS * BL, (q + 1) * QSTEPS * BL)
            tsl = slice(q * QSTEPS, (q + 1) * QSTEPS)
            ps = pre_ps.tile([128, QSTEPS * BL], FP32, tag="bps")
            if m < 4:
                lhsT = wx_rmu[:, m * 128:(m + 1) * 128]
            else:
                lhsT = wx_h[:, (m - 4) * 128:(m - 4 + 1) * 128]
            nc.tensor.matmul(ps, lhsT, cur_x[:, nsl], start=True, stop=True)
            src = ps.rearrange("p (t g b) -> p t g b", g=2, b=GB)
            dest = cur_gx[:, tsl, :, m, :]
            if m < 4:
                nc.vector.tensor_scalar_add(dest, src, b_rmu[:, m:m + 1])
            else:
                nc.scalar.activation(dest, src, AF.Identity,
                                     bias=b_h[:, m - 4:m - 4 + 1])

        # precompute-unit emission schedule within the previous chunk
        sched = {}
        for u in range(24):
            sched.setdefault(2 + (u * 5) // 2, []).append(u)

        # initial state (= beta_0 * h_{-1} = 0)
        hb = spool.tile([128, 2, 2, GB], BF16, tag="hb")
        nc.vector.memset(hb.rearrange("p a k b -> p (a k b)"), 0.0)

        # chunk 0 x + precompute fully upfront
        cur_x = xpool.tile([128, C * BL], BF16, tag="xch")
        nc.sync.dma_start(cur_x, xT[:, 0:C * BL])
        cur_gx = gxpool.tile([128, C, 2, 6, GB], BF16, tag="gx")
        for u in range(24):
            emit_pre_unit(0, u)

        # Per-group state for the skewed two-group software pipeline.
        hb_g = [hb[:, 0], hb[:, 1]]
        st = [dict(state_parts=(hb[:, 0],)), dict(state_parts=(hb[:, 1],))]
        gx_of = {}

        def front(g, t):
            """Step front half for group g. The r/mu matmuls consume the
            state as the unmaterialized pair (p2, e2) from the previous
            back-half — Wh@p2 + Wh@e2 accumulates to Wh@hb exactly, and the
            p2 matmuls fire before the previous tanh has even finished."""
            s = st[g]
            bet_t = bet_all[:, t, g]           # [p, k, b]
            last = (t == T - 1)
            gx = gx_of[t // C]

            sps = sps_pool.tile([128, 6, GB], FP32, tag=f"s{g}",
                                name=f"sps{g}")
            s["sps"] = sps
            s["hb"] = hb_g[g]

            nc.tensor.matmul(sps, ident[:, :], gx[:, t % C, g],
                             start=True, stop=False)
            for part in s["state_parts"]:
                for m in range(4):
                    for k in range(2):
                        nc.tensor.matmul(sps[:, m, :],
                                         wh_all[:, m * 2 + k, :],
                                         part[:, k, :], start=False,
                                         stop=False)

            rm = spool.tile([128, 4, GB], BF16, tag=f"rm{g}", name=f"rm{g}")
            nc.scalar.activation(rm, sps[:, 0:4, :], AF.Sigmoid)

            rh = spool.tile([128, 2, GB], BF16, tag=f"rh{g}", name=f"rh{g}")
            nc.vector.tensor_mul(rh, rm[:, 0:2, :], s["hb"])
            s["rh"] = rh

            # p = beta*hb early on DVE; after mu: p2 = (1-mu)*p (2 Pool ops)
            # and w = beta*mu for the e2 = w*hhat tail op.
            p_t = spool.tile([128, 2, GB], BF16, tag=f"p{g}", name=f"p{g}")
            t1 = spool.tile([128, 2, GB], BF16, tag=f"t1{g}", name=f"t1{g}")
            p2 = spool.tile([128, 2, GB], BF16, tag=f"p2{g}", name=f"p2{g}")
            w_t = spool.tile([128, 2, GB], BF16, tag=f"w{g}", name=f"w{g}")
            nc.gpsimd.tensor_scalar(t1, rm[:, 2:4, :], -1.0, 1.0,
                                    ALU.mult, ALU.add)
            if not last:
                nc.vector.tensor_mul(p_t, bet_t, s["hb"])
                nc.gpsimd.tensor_mul(p2, t1, p_t)
                nc.gpsimd.tensor_mul(w_t, bet_t, rm[:, 2:4, :])
                s["w"] = w_t
            else:
                # h_out = (1-mu)*hb + mu*hhat
                nc.gpsimd.tensor_mul(p2, t1, s["hb"])
                s["w"] = rm[:, 2:4, :]
            s["p2"] = p2

        def back(g, t):
            """Step back half for group g: h matmuls, tanh, e2 = w*hhat;
            the state is materialized off the critical path."""
            s = st[g]
            last = (t == T - 1)
            sps, rh = s["sps"], s["rh"]

            for k in range(2):
                for m in range(2):
                    nc.tensor.matmul(sps[:, 4 + m, :],
                                     wh_all[:, 8 + m * 2 + k, :],
                                     rh[:, k, :], start=False,
                                     stop=(m == 1 and k == 1))

            hhat = spool.tile([128, 2, GB], BF16, tag=f"hh{g}",
                              name=f"hh{g}")
            nc.scalar.activation(hhat, sps[:, 4:6, :], AF.Tanh)

            e_g = spool.tile([128, 2, GB], BF16, tag=f"e{g}", name=f"e{g}")
            nc.vector.tensor_mul(e_g, s["w"], hhat)
            if not last:
                s["state_parts"] = (s["p2"], e_g)
                # materialized state for rh/p of the next step (off-path)
                nhb = spool.tile([128, 2, GB], BF16, tag=f"hb{g}",
                                 name=f"nhb{g}")
                nc.vector.tensor_add(nhb, s["p2"], e_g)
                hb_g[g] = nhb
            else:
                hout = spool.tile([128, 2, GB], FP32, tag=f"ho{g}",
                                  name=f"ho{g}")
                nc.vector.tensor_add(hout, s["p2"], e_g)
                nc.sync.dma_start(out_d[:, g * BL:(g + 1) * BL],
                                  hout.rearrange("p k b -> p (k b)"))

        gx_of[0] = cur_gx
        for c in range(NCHUNK):
            if c + 1 < NCHUNK:
                cur_x = xpool.tile([128, C * BL], BF16, tag="xch")
                nc.sync.dma_start(cur_x,
                                  xT[:, (c + 1) * C * BL:(c + 2) * C * BL])
                cur_gx = gxpool.tile([128, C, 2, 6, GB], BF16, tag="gx")
                gx_of[c + 1] = cur_gx

            for i in range(C):
                t = c * C + i
                # skewed pipeline: B runs half a step behind A
                front(0, t)
                if t > 0:
                    back(1, t - 1)
                front(1, t)
                back(0, t)

                # spread next chunk's precompute over this chunk's steps
                if c + 1 < NCHUNK and i in sched:
                    for u in sched[i]:
                        emit_pre_unit(c + 1, u)

        back(1, T - 1)

    nc.compile()
    return nc


def _prep_inputs(x, delta, W_mu, b_mu, W_r, b_r, W_h, b_h, W_td, b_td):
    bf = ml_dtypes.bfloat16
    # weights: first H rows act on h, last D rows act on x
    wh_gates = [W_r[:H], W_mu[:H], W_h[:H]]          # each [256, 256]
    wx_rmu = np.concatenate([W_r[H:], W_mu[H:]], axis=1)      # [128, 512]
    wx_h = W_h[H:]

    # contiguous [128,128] weight tiles, gate-major, k (contraction) minor
    tiles = []
    for gi, m in ((0, 0), (0, 1), (1, 0), (1, 1), (2, 0), (2, 1)):
        for k in range(2):
            tiles.append(wh_gates[gi][k * 128:(k + 1) * 128,
                                      m * 128:(m + 1) * 128])
    wh_all = np.concatenate(tiles, axis=1)                    # [128, 1536]

    def pcol(v):  # [2*128] -> [128, 2] column-per-tile
        return np.ascontiguousarray(np.stack([v[:128], v[128:]], axis=1),
                                    dtype=np.float32)

    b_rmu_col = np.concatenate([b_r, b_mu])                    # [512]
    b_rmu_t = np.ascontiguousarray(
        np.stack([b_rmu_col[i * 128:(i + 1) * 128] for i in range(4)], axis=1),
        dtype=np.float32)                                      # [128, 4]

    shared = {
        "wx_rmu": np.ascontiguousarray(wx_rmu, dtype=bf),
        "wx_h": np.ascontiguousarray(wx_h, dtype=bf),
        "wtd": np.ascontiguousarray(W_td, dtype=bf),
        "wh_all": np.ascontiguousarray(wh_all, dtype=bf),
        "b_rmu": b_rmu_t,
        "b_h": pcol(b_h),
        "nb_td": pcol(-b_td),
    }

    # delta shifted by one step: beta used at step t is beta(t+1)
    dshift = np.concatenate(
        [delta[:, 1:, :], np.zeros((B, 1, D), np.float32)], axis=1)

    in_maps = []
    for ci in range(NCORES):
        xs = x[ci * BL:(ci + 1) * BL]          # [32, 512, 128]
        ds = dshift[ci * BL:(ci + 1) * BL]
        # [BL, T, D] -> [D, T, BL] -> [D, T*BL]  (column t*BL + b)
        xt = np.ascontiguousarray(
            xs.transpose(2, 1, 0).reshape(D, T * BL), dtype=bf)
        dt_ = np.ascontiguousarray(
            ds.transpose(2, 1, 0).reshape(D, T * BL), dtype=bf)
        in_maps.append({"xT": xt, "dTs": dt_, **shared})
    return in_maps


def kernel(x, delta, W_mu, b_mu, W_r, b_r, W_h, b_h, W_td, b_td):
    args = tuple(np.asarray(a, dtype=np.float32) for a in
                 (x, delta, W_mu, b_mu, W_r, b_r, W_h, b_h, W_td, b_td))
    in_maps = _prep_inputs(*args)
    if "nc" not in _cache:
        _cache["nc"] = _build()
    res = run_bass_kernel_spmd(_cache["nc"], in_maps,
                               core_ids=list(range(NCORES)))
    out = np.empty((B, H), np.float32)
    for ci in range(NCORES):
        o = res.results[ci]["hT_out"]          # [128, 2*BL]; col = g*32+k*16+b
        for g in range(2):
            for k in range(2):
                out[ci * BL + g * GB:ci * BL + (g + 1) * GB,
                    k * 128:(k + 1) * 128] = \
                    o[:, g * 32 + k * GB:g * 32 + (k + 1) * GB].T
    return out


# revision 16
# speedup vs baseline: 1.0011x; 1.0011x over previous
"""GRU-D style GRUI encoder kernel for Trainium2 (Bass/Tile), 8 NeuronCores.

Data-parallel over batch B=256 across 8 cores (32 seqs/core), with the 32
sequences split into two groups A/B of 16 that are software-pipelined at
sub-step granularity: while group A's recurrence chain is in its ACT/DVE
phase, group B's matmuls run on the PE, halving the effective per-step
dependency-chain latency.

Other structure (informed by trace analysis):
  - beta = exp(-relu(delta@Wtd+b)) for ALL T steps is computed upfront
    (no ACT-table thrash between Exp and Sigmoid in the steady loop).
  - per step each group has ONE psum region [128, 6, 16] = [r0 r1 m0 m1
    h0 h1], seeded by a single identity-inject matmul.
  - r|mu sigmoid merged into one ACT op per group (ACT op count is the
    per-step budget limit); tanh separate.
  - state update hb' = p + w*(hhat-hb) with p = beta*hb computed on
    GPSIMD right at step start and w = beta*mu after mu — both off the
    critical path; the tail after tanh is 3 short DVE ops per group.
  - gate-x GEMMs for chunk c+1 are spread one-per-~2.5-steps across
    chunk c's emission so the PE never sees a burst at chunk boundaries.
"""

import numpy as np
import ml_dtypes
from contextlib import ExitStack

import concourse.bass as bass
import concourse.bacc as bacc
import concourse.tile as tile
from concourse import mybir
from concourse.bass_utils import run_bass_kernel_spmd
from concourse.masks import make_identity

B, T, D, H = 256, 512, 128, 256
NCORES = 8
BL = B // NCORES          # 32 sequences per core
GB = 16                   # sequences per pipeline group (2 groups)
C = 64                    # steps per chunk
NCHUNK = T // C
QSTEPS = 16               # steps per precompute GEMM group (N = 16*32 = 512)

FP32 = mybir.dt.float32
BF16 = mybir.dt.bfloat16
AF = mybir.ActivationFunctionType
ALU = mybir.AluOpType

_cache = {}


def _build():
    nc = bacc.Bacc("TRN2", target_bir_lowering=False, debug=False,
                   num_devices=NCORES)

    xT = nc.dram_tensor("xT", [D, T * BL], BF16, kind="ExternalInput")
    dTs = nc.dram_tensor("dTs", [D, T * BL], BF16, kind="ExternalInput")
    wx_rmu_d = nc.dram_tensor("wx_rmu", [D, 2 * H], BF16, kind="ExternalInput")
    wx_h_d = nc.dram_tensor("wx_h", [D, H], BF16, kind="ExternalInput")
    wtd_d = nc.dram_tensor("wtd", [D, H], BF16, kind="ExternalInput")
    wh_all_d = nc.dram_tensor("wh_all", [128, 12 * 128], BF16,
                              kind="ExternalInput")
    b_rmu_d = nc.dram_tensor("b_rmu", [128, 4], FP32, kind="ExternalInput")
    b_h_d = nc.dram_tensor("b_h", [128, 2], FP32, kind="ExternalInput")
    nb_td_d = nc.dram_tensor("nb_td", [128, 2], FP32, kind="ExternalInput")
    out_d = nc.dram_tensor("hT_out", [128, 2 * BL], FP32, kind="ExternalOutput")

    with ExitStack() as ctx:
        tc = ctx.enter_context(tile.TileContext(nc))
        wpool = ctx.enter_context(tc.tile_pool(name="weights", bufs=1))
        betapool = ctx.enter_context(tc.tile_pool(name="beta", bufs=1))
        xpool = ctx.enter_context(tc.tile_pool(name="xin", bufs=2))
        gxpool = ctx.enter_context(tc.tile_pool(name="gx", bufs=2))
        pre_ps = ctx.enter_context(tc.tile_pool(name="pre_ps", bufs=2,
                                                space="PSUM"))
        sps_pool = ctx.enter_context(tc.tile_pool(name="sps", bufs=2,
                                                  space="PSUM"))
        spool = ctx.enter_context(tc.tile_pool(name="state", bufs=3))

        # --- weights / constants into SBUF ---
        wh_all = wpool.tile([128, 12, 128], BF16)
        nc.sync.dma_start(wh_all.rearrange("p a b -> p (a b)"), wh_all_d[:, :])
        wx_rmu = wpool.tile([128, 2 * H], BF16)
        nc.sync.dma_start(wx_rmu, wx_rmu_d[:, :])
        wx_h = wpool.tile([128, H], BF16)
        nc.sync.dma_start(wx_h, wx_h_d[:, :])
        wtd = wpool.tile([128, H], BF16)
        nc.sync.dma_start(wtd, wtd_d[:, :])
        b_rmu = wpool.tile([128, 4], FP32)
        nc.sync.dma_start(b_rmu, b_rmu_d[:, :])
        b_h = wpool.tile([128, 2], FP32)
        nc.sync.dma_start(b_h, b_h_d[:, :])
        nb_td = wpool.tile([128, 2], FP32)
        nc.sync.dma_start(nb_td, nb_td_d[:, :])
        ident = wpool.tile([128, 128], BF16)
        make_identity(nc, ident)

        # Touch bias tiles from DVE once so later TSP/ACT consumers don't
        # carry the DMA wait (walrus rejects TSP with 2 sync waits).
        scratch = wpool.tile([128, 4], FP32, tag="scratch")
        nc.vector.tensor_copy(scratch, b_rmu)
        scratch2 = wpool.tile([128, 2], FP32, tag="scratch2")
        nc.vector.tensor_copy(scratch2, b_h)
        scratch3 = wpool.tile([128, 2], FP32, tag="scratch3")
        nc.vector.tensor_copy(scratch3, nb_td)

        # beta for every step: [p, t, gb, k, b]
        bet_all = betapool.tile([128, T, 2, 2, GB], BF16)

        # ---------- Phase 2 helper (also used during Phase 1) ----------
        def emit_pre_unit(c, u):
            """One x-GEMM + evacuation for chunk c, unit u (0..23)."""
            q, m = divmod(u, 6)
            nsl = slice(q * QSTEPS * BL, (q + 1) * QSTEPS * BL)
            tsl = slice(q * QSTEPS, (q + 1) * QSTEPS)
            ps = pre_ps.tile([128, QSTEPS * BL], FP32, tag="bps")
            if m < 4:
                lhsT = wx_rmu[:, m * 128:(m + 1) * 128]
            else:
                lhsT = wx_h[:, (m - 4) * 128:(m - 4 + 1) * 128]
            nc.tensor.matmul(ps, lhsT, cur_x[:, nsl], start=True, stop=True)
            src = ps.rearrange("p (t g b) -> p t g b", g=2, b=GB)
            dest = cur_gx[:, tsl, :, m, :]
            if m < 4:
                nc.vector.tensor_scalar_add(dest, src, b_rmu[:, m:m + 1])
            else:
                nc.scalar.activation(dest, src, AF.Identity,
                                     bias=b_h[:, m - 4:m - 4 + 1])

        # chunk-0 x + gx tiles so chunk 0's precompute can interleave with
        # the beta phase (PE/DVE fill in while ACT runs the Exp ops; the
        # Identity-bias evacuations live in every ACT table, so no reload)
        cur_x = xpool.tile([128, C * BL], BF16, tag="xch")
        nc.sync.dma_start(cur_x, xT[:, 0:C * BL])
        cur_gx = gxpool.tile([128, C, 2, 6, GB], BF16, tag="gx")

        # ---------- Phase 1: all temporal-decay betas upfront ----------
        for c in range(NCHUNK):
            dch = xpool.tile([128, C * BL], BF16, tag="dch")
            nc.sync.dma_start(dch, dTs[:, c * C * BL:(c + 1) * C * BL])
            for q in range(C // QSTEPS):
                nsl = slice(q * QSTEPS * BL, (q + 1) * QSTEPS * BL)
                for k in range(2):
                    bps = pre_ps.tile([128, QSTEPS * BL], FP32, tag="bps")
                    nc.tensor.matmul(bps, wtd[:, k * 128:(k + 1) * 128],
                                     dch[:, nsl], start=True, stop=True)
                    # exp(-(z + b)) ; clamped to <=1 below (== exp(-relu))
                    nc.scalar.activation(
                        bet_all[:, c * C + q * QSTEPS:
                                c * C + (q + 1) * QSTEPS, :, k, :],
                        bps.rearrange("p (t g b) -> p t g b", g=2, b=GB),
                        AF.Exp, bias=nb_td[:, k:k + 1], scale=-1.0)
            nc.vector.tensor_scalar_min(
                bet_all[:, c * C:(c + 1) * C].rearrange(
                    "p t g k b -> p (t g k b)"),
                bet_all[:, c * C:(c + 1) * C].rearrange(
                    "p t g k b -> p (t g k b)"), 1.0)
            # interleave chunk 0's gate-x precompute with the beta phase
            for u in range(3 * c, 3 * c + 3):
                emit_pre_unit(0, u)

        # ---------- Phase 2: recurrence ----------
        # precompute-unit emission schedule within the previous chunk
        sched = {}
        for u in range(24):
            sched.setdefault(2 + (u * 5) // 2, []).append(u)

        # initial state (= beta_0 * h_{-1} = 0)
        hb = spool.tile([128, 2, 2, GB], BF16, tag="hb")
        nc.vector.memset(hb.rearrange("p a k b -> p (a k b)"), 0.0)

        # Per-group state for the skewed two-group software pipeline.
        hb_g = [hb[:, 0], hb[:, 1]]
        st = [dict(), dict()]
        gx_of = {}

        def front(g, t):
            """Step front half for group g: p, inject, r/mu matmuls,
            sigmoid, rh, w."""
            s = st[g]
            bet_t = bet_all[:, t, g]           # [p, k, b]
            last = (t == T - 1)
            gx = gx_of[t // C]

            sps = sps_pool.tile([128, 6, GB], FP32, tag=f"s{g}",
                                name=f"sps{g}")
            s["sps"] = sps
            s["hb"] = hb_g[g]

            p_t = spool.tile([128, 2, GB], BF16, tag=f"p{g}", name=f"p{g}")
            if not last:
                nc.vector.tensor_mul(p_t, bet_t, s["hb"])
            s["p"] = p_t

            nc.tensor.matmul(sps, ident[:, :], gx[:, t % C, g],
                             start=True, stop=False)
            for m in range(4):
                for k in range(2):
                    nc.tensor.matmul(sps[:, m, :], wh_all[:, m * 2 + k, :],
                                     s["hb"][:, k, :], start=False,
                                     stop=False)

            rm = spool.tile([128, 4, GB], BF16, tag=f"rm{g}", name=f"rm{g}")
            nc.scalar.activation(rm, sps[:, 0:4, :], AF.Sigmoid)

            rh = spool.tile([128, 2, GB], BF16, tag=f"rh{g}", name=f"rh{g}")
            nc.vector.tensor_mul(rh, rm[:, 0:2, :], s["hb"])
            s["rh"] = rh

            # w = beta*mu, then p2 = p - w*hb so the post-tanh tail is only
            # two dependent DVE ops (e = w*hhat; hb' = p2 + e)
            w_t = spool.tile([128, 2, GB], BF16, tag=f"w{g}", name=f"w{g}")
            f_t = spool.tile([128, 2, GB], BF16, tag=f"f{g}", name=f"f{g}")
            p2 = spool.tile([128, 2, GB], BF16, tag=f"p2{g}", name=f"p2{g}")
            if not last:
                nc.gpsimd.tensor_mul(w_t, bet_t, rm[:, 2:4, :])
                nc.gpsimd.tensor_mul(f_t, w_t, s["hb"])
                nc.gpsimd.tensor_sub(p2, p_t, f_t)
            else:
                # h_out = (hb - mu*hb) + mu*hhat
                nc.gpsimd.tensor_copy(w_t, rm[:, 2:4, :])
                nc.gpsimd.tensor_mul(f_t, w_t, s["hb"])
                nc.gpsimd.tensor_sub(p2, s["hb"], f_t)
            s["w"] = w_t
            s["p2"] = p2

        def back(g, t):
            """Step back half for group g: h matmuls, tanh, state update."""
            s = st[g]
            last = (t == T - 1)
            sps, rh = s["sps"], s["rh"]

            for k in range(2):
                for m in range(2):
                    nc.tensor.matmul(sps[:, 4 + m, :],
                                     wh_all[:, 8 + m * 2 + k, :],
                                     rh[:, k, :], start=False,
                                     stop=(m == 1 and k == 1))

            hhat = spool.tile([128, 2, GB], BF16, tag=f"hh{g}",
                              name=f"hh{g}")
            nc.scalar.activation(hhat, sps[:, 4:6, :], AF.Tanh)

            e_g = spool.tile([128, 2, GB], BF16, tag=f"e{g}", name=f"e{g}")
            nc.vector.tensor_mul(e_g, s["w"], hhat)
            if not last:
                nhb = spool.tile([128, 2, GB], BF16, tag=f"hb{g}",
                                 name=f"nhb{g}")
                nc.vector.tensor_add(nhb, s["p2"], e_g)
                hb_g[g] = nhb
            else:
                hout = spool.tile([128, 2, GB], FP32, tag=f"ho{g}",
                                  name=f"ho{g}")
                nc.vector.tensor_add(hout, s["p2"], e_g)
                nc.sync.dma_start(out_d[:, g * BL:(g + 1) * BL],
                                  hout.rearrange("p k b -> p (k b)"))

        gx_of[0] = cur_gx
        for c in range(NCHUNK):
            if c + 1 < NCHUNK:
                cur_x = xpool.tile([128, C * BL], BF16, tag="xch")
                nc.sync.dma_start(cur_x,
                                  xT[:, (c + 1) * C * BL:(c + 2) * C * BL])
                cur_gx = gxpool.tile([128, C, 2, 6, GB], BF16, tag="gx")
                gx_of[c + 1] = cur_gx

            for i in range(C):
                t = c * C + i
                # skewed pipeline: B runs half a step behind A
                front(0, t)
                if t > 0:
                    back(1, t - 1)
                front(1, t)
                back(0, t)

                # spread next chunk's precompute over this chunk's steps
                if c + 1 < NCHUNK and i in sched:
                    for u in sched[i]:
                        emit_pre_unit(c + 1, u)

        back(1, T - 1)

    nc.compile()
    return nc


def _prep_inputs(x, delta, W_mu, b_mu, W_r, b_r, W_h, b_h, W_td, b_td):
    bf = ml_dtypes.bfloat16
    # weights: first H rows act on h, last D rows act on x
    wh_gates = [W_r[:H], W_mu[:H], W_h[:H]]          # each [256, 256]
    wx_rmu = np.concatenate([W_r[H:], W_mu[H:]], axis=1)      # [128, 512]
    wx_h = W_h[H:]

    # contiguous [128,128] weight tiles, gate-major, k (contraction) minor
    tiles = []
    for gi, m in ((0, 0), (0, 1), (1, 0), (1, 1), (2, 0), (2, 1)):
        for k in range(2):
            tiles.append(wh_gates[gi][k * 128:(k + 1) * 128,
                                      m * 128:(m + 1) * 128])
    wh_all = np.concatenate(tiles, axis=1)                    # [128, 1536]

    def pcol(v):  # [2*128] -> [128, 2] column-per-tile
        return np.ascontiguousarray(np.stack([v[:128], v[128:]], axis=1),
                                    dtype=np.float32)

    b_rmu_col = np.concatenate([b_r, b_mu])                    # [512]
    b_rmu_t = np.ascontiguousarray(
        np.stack([b_rmu_col[i * 128:(i + 1) * 128] for i in range(4)], axis=1),
        dtype=np.float32)                                      # [128, 4]

    shared = {
        "wx_rmu": np.ascontiguousarray(wx_rmu, dtype=bf),
        "wx_h": np.ascontiguousarray(wx_h, dtype=bf),
        "wtd": np.ascontiguousarray(W_td, dtype=bf),
        "wh_all": np.ascontiguousarray(wh_all, dtype=bf),
        "b_rmu": b_rmu_t,
        "b_h": pcol(b_h),
        "nb_td": pcol(-b_td),
    }

    # delta shifted by one step: beta used at step t is beta(t+1)
    dshift = np.concatenate(
        [delta[:, 1:, :], np.zeros((B, 1, D), np.float32)], axis=1)

    in_maps = []
    for ci in range(NCORES):
        xs = x[ci * BL:(ci + 1) * BL]          # [32, 512, 128]
        ds = dshift[ci * BL:(ci + 1) * BL]
        # [BL, T, D] -> [D, T, BL] -> [D, T*BL]  (column t*BL + b)
        xt = np.ascontiguousarray(
            xs.transpose(2, 1, 0).reshape(D, T * BL), dtype=bf)
        dt_ = np.ascontiguousarray(
            ds.transpose(2, 1, 0).reshape(D, T * BL), dtype=bf)
        in_maps.append({"xT": xt, "dTs": dt_, **shared})
    return in_maps


def kernel(x, delta, W_mu, b_mu, W_r, b_r, W_h, b_h, W_td, b_td):
    args = tuple(np.asarray(a, dtype=np.float32) for a in
                 (x, delta, W_mu, b_mu, W_r, b_r, W_h, b_h, W_td, b_td))
    in_maps = _prep_inputs(*args)
    if "nc" not in _cache:
        _cache["nc"] = _build()
    res = run_bass_kernel_spmd(_cache["nc"], in_maps,
                               core_ids=list(range(NCORES)))
    out = np.empty((B, H), np.float32)
    for ci in range(NCORES):
        o = res.results[ci]["hT_out"]          # [128, 2*BL]; col = g*32+k*16+b
        for g in range(2):
            for k in range(2):
                out[ci * BL + g * GB:ci * BL + (g + 1) * GB,
                    k * 128:(k + 1) * 128] = \
                    o[:, g * 32 + k * GB:g * 32 + (k + 1) * GB].T
    return out


# revision 18
# speedup vs baseline: 1.0133x; 1.0122x over previous
"""GRU-D style GRUI encoder kernel for Trainium2 (Bass/Tile), 8 NeuronCores.

Data-parallel over batch B=256 across 8 cores (32 seqs/core), with the 32
sequences split into two groups A/B of 16 that are software-pipelined at
sub-step granularity: while group A's recurrence chain is in its ACT/DVE
phase, group B's matmuls run on the PE, halving the effective per-step
dependency-chain latency.

Other structure (informed by trace analysis):
  - beta = exp(-relu(delta@Wtd+b)) for ALL T steps is computed upfront
    (no ACT-table thrash between Exp and Sigmoid in the steady loop).
  - per step each group has ONE psum region [128, 6, 16] = [r0 r1 m0 m1
    h0 h1], seeded by a single identity-inject matmul.
  - r|mu sigmoid merged into one ACT op per group (ACT op count is the
    per-step budget limit); tanh separate.
  - state update hb' = p + w*(hhat-hb) with p = beta*hb computed on
    GPSIMD right at step start and w = beta*mu after mu — both off the
    critical path; the tail after tanh is 3 short DVE ops per group.
  - gate-x GEMMs for chunk c+1 are spread one-per-~2.5-steps across
    chunk c's emission so the PE never sees a burst at chunk boundaries.
"""

import numpy as np
import ml_dtypes
from contextlib import ExitStack

import concourse.bass as bass
import concourse.bacc as bacc
import concourse.tile as tile
from concourse import mybir
from concourse.bass_utils import run_bass_kernel_spmd
from concourse.masks import make_identity

B, T, D, H = 256, 512, 128, 256
NCORES = 8
BL = B // NCORES          # 32 sequences per core
GB = 16                   # sequences per pipeline group (2 groups)
C = 64                    # steps per chunk
NCHUNK = T // C
QSTEPS = 16               # steps per precompute GEMM group (N = 16*32 = 512)

FP32 = mybir.dt.float32
BF16 = mybir.dt.bfloat16
AF = mybir.ActivationFunctionType
ALU = mybir.AluOpType

_cache = {}


def _build():
    nc = bacc.Bacc("TRN2", target_bir_lowering=False, debug=False,
                   num_devices=NCORES)

    xT = nc.dram_tensor("xT", [D, T * BL], BF16, kind="ExternalInput")
    dTs = nc.dram_tensor("dTs", [D, T * BL], BF16, kind="ExternalInput")
    wx_rmu_d = nc.dram_tensor("wx_rmu", [D, 2 * H], BF16, kind="ExternalInput")
    wx_h_d = nc.dram_tensor("wx_h", [D, H], BF16, kind="ExternalInput")
    wtd_d = nc.dram_tensor("wtd", [D, H], BF16, kind="ExternalInput")
    wh_all_d = nc.dram_tensor("wh_all", [128, 12 * 128], BF16,
                              kind="ExternalInput")
    b_rmu_d = nc.dram_tensor("b_rmu", [128, 4], FP32, kind="ExternalInput")
    b_h_d = nc.dram_tensor("b_h", [128, 2], FP32, kind="ExternalInput")
    nb_td_d = nc.dram_tensor("nb_td", [128, 2], FP32, kind="ExternalInput")
    out_d = nc.dram_tensor("hT_out", [128, 2 * BL], FP32, kind="ExternalOutput")

    with ExitStack() as ctx:
        tc = ctx.enter_context(tile.TileContext(nc))
        wpool = ctx.enter_context(tc.tile_pool(name="weights", bufs=1))
        betapool = ctx.enter_context(tc.tile_pool(name="beta", bufs=1))
        xpool = ctx.enter_context(tc.tile_pool(name="xin", bufs=2))
        gxpool = ctx.enter_context(tc.tile_pool(name="gx", bufs=2))
        pre_ps = ctx.enter_context(tc.tile_pool(name="pre_ps", bufs=2,
                                                space="PSUM"))
        sps_pool = ctx.enter_context(tc.tile_pool(name="sps", bufs=2,
                                                  space="PSUM"))
        spool = ctx.enter_context(tc.tile_pool(name="state", bufs=3))

        # --- weights / constants into SBUF ---
        wh_all = wpool.tile([128, 12, 128], BF16)
        nc.sync.dma_start(wh_all.rearrange("p a b -> p (a b)"), wh_all_d[:, :])
        wx_rmu = wpool.tile([128, 2 * H], BF16)
        nc.sync.dma_start(wx_rmu, wx_rmu_d[:, :])
        wx_h = wpool.tile([128, H], BF16)
        nc.sync.dma_start(wx_h, wx_h_d[:, :])
        wtd = wpool.tile([128, H], BF16)
        nc.sync.dma_start(wtd, wtd_d[:, :])
        b_rmu = wpool.tile([128, 4], FP32)
        nc.sync.dma_start(b_rmu, b_rmu_d[:, :])
        b_h = wpool.tile([128, 2], FP32)
        nc.sync.dma_start(b_h, b_h_d[:, :])
        nb_td = wpool.tile([128, 2], FP32)
        nc.sync.dma_start(nb_td, nb_td_d[:, :])
        ident = wpool.tile([128, 128], BF16)
        make_identity(nc, ident)

        # Touch bias tiles from DVE once so later TSP/ACT consumers don't
        # carry the DMA wait (walrus rejects TSP with 2 sync waits).
        scratch = wpool.tile([128, 4], FP32, tag="scratch")
        nc.vector.tensor_copy(scratch, b_rmu)
        scratch2 = wpool.tile([128, 2], FP32, tag="scratch2")
        nc.vector.tensor_copy(scratch2, b_h)
        scratch3 = wpool.tile([128, 2], FP32, tag="scratch3")
        nc.vector.tensor_copy(scratch3, nb_td)

        # beta for every step: [p, t, gb, k, b]
        bet_all = betapool.tile([128, T, 2, 2, GB], BF16)

        # ---------- Phase 1: all temporal-decay betas upfront ----------
        for c in range(NCHUNK):
            dch = xpool.tile([128, C * BL], BF16, tag="dch")
            nc.sync.dma_start(dch, dTs[:, c * C * BL:(c + 1) * C * BL])
            for q in range(C // QSTEPS):
                nsl = slice(q * QSTEPS * BL, (q + 1) * QSTEPS * BL)
                for k in range(2):
                    bps = pre_ps.tile([128, QSTEPS * BL], FP32, tag="bps")
                    nc.tensor.matmul(bps, wtd[:, k * 128:(k + 1) * 128],
                                     dch[:, nsl], start=True, stop=True)
                    # exp(-(z + b)) ; clamped to <=1 below (== exp(-relu))
                    nc.scalar.activation(
                        bet_all[:, c * C + q * QSTEPS:
                                c * C + (q + 1) * QSTEPS, :, k, :],
                        bps.rearrange("p (t g b) -> p t g b", g=2, b=GB),
                        AF.Exp, bias=nb_td[:, k:k + 1], scale=-1.0)
            nc.vector.tensor_scalar_min(
                bet_all[:, c * C:(c + 1) * C].rearrange(
                    "p t g k b -> p (t g k b)"),
                bet_all[:, c * C:(c + 1) * C].rearrange(
                    "p t g k b -> p (t g k b)"), 1.0)

        # ---------- Phase 2: gate-x precompute + recurrence ----------
        def emit_pre_unit(c, u):
            """One x-GEMM + evacuation for chunk c, unit u (0..23)."""
            q, m = divmod(u, 6)
            nsl = slice(q * QSTEPS * BL, (q + 1) * QSTEPS * BL)
            tsl = slice(q * QSTEPS, (q + 1) * QSTEPS)
            ps = pre_ps.tile([128, QSTEPS * BL], FP32, tag="bps")
            if m < 4:
                lhsT = wx_rmu[:, m * 128:(m + 1) * 128]
            else:
                lhsT = wx_h[:, (m - 4) * 128:(m - 4 + 1) * 128]
            nc.tensor.matmul(ps, lhsT, cur_x[:, nsl], start=True, stop=True)
            src = ps.rearrange("p (t g b) -> p t g b", g=2, b=GB)
            dest = cur_gx[:, tsl, :, m, :]
            if m < 4:
                nc.vector.tensor_scalar_add(dest, src, b_rmu[:, m:m + 1])
            else:
                nc.scalar.activation(dest, src, AF.Identity,
                                     bias=b_h[:, m - 4:m - 4 + 1])

        # precompute-unit emission schedule within the previous chunk
        sched = {}
        for u in range(24):
            sched.setdefault(2 + (u * 5) // 2, []).append(u)

        # initial state (= beta_0 * h_{-1} = 0)
        hb = spool.tile([128, 2, 2, GB], BF16, tag="hb")
        nc.vector.memset(hb.rearrange("p a k b -> p (a k b)"), 0.0)

        # chunk 0 x + precompute fully upfront
        cur_x = xpool.tile([128, C * BL], BF16, tag="xch")
        nc.sync.dma_start(cur_x, xT[:, 0:C * BL])
        cur_gx = gxpool.tile([128, C, 2, 6, GB], BF16, tag="gx")
        for u in range(24):
            emit_pre_unit(0, u)

        # Per-group state for the skewed two-group software pipeline.
        hb_g = [hb[:, 0], hb[:, 1]]
        st = [dict(), dict()]
        gx_of = {}

        def front(g, t):
            """Step front half for group g: p, inject, r/mu matmuls,
            sigmoid, rh, w."""
            s = st[g]
            bet_t = bet_all[:, t, g]           # [p, k, b]
            last = (t == T - 1)
            gx = gx_of[t // C]

            sps = sps_pool.tile([128, 6, GB], FP32, tag=f"s{g}",
                                name=f"sps{g}")
            s["sps"] = sps
            s["hb"] = hb_g[g]

            p_t = spool.tile([128, 2, GB], BF16, tag=f"p{g}", name=f"p{g}")
            if not last:
                nc.vector.tensor_mul(p_t, bet_t, s["hb"])
            s["p"] = p_t

            nc.tensor.matmul(sps, ident[:, :], gx[:, t % C, g],
                             start=True, stop=False)
            for m in range(4):
                for k in range(2):
                    nc.tensor.matmul(sps[:, m, :], wh_all[:, m * 2 + k, :],
                                     s["hb"][:, k, :], start=False,
                                     stop=False)

            rm = spool.tile([128, 4, GB], BF16, tag=f"rm{g}", name=f"rm{g}")
            nc.scalar.activation(rm, sps[:, 0:4, :], AF.Sigmoid)

            rh = spool.tile([128, 2, GB], BF16, tag=f"rh{g}", name=f"rh{g}")
            nc.vector.tensor_mul(rh, rm[:, 0:2, :], s["hb"])
            s["rh"] = rh

            # t1 = 1-mu fires straight off the sigmoid (no wait on w), so
            # p2 = t1*p arrives well before the post-tanh tail needs it;
            # w = beta*mu only has to beat e = w*hhat (tanh + one sem).
            t1 = spool.tile([128, 2, GB], BF16, tag=f"t1{g}", name=f"t1{g}")
            w_t = spool.tile([128, 2, GB], BF16, tag=f"w{g}", name=f"w{g}")
            p2 = spool.tile([128, 2, GB], BF16, tag=f"p2{g}", name=f"p2{g}")
            nc.gpsimd.tensor_scalar(t1, rm[:, 2:4, :], -1.0, 1.0,
                                    ALU.mult, ALU.add)
            if not last:
                nc.gpsimd.tensor_mul(w_t, bet_t, rm[:, 2:4, :])
                nc.gpsimd.tensor_mul(p2, t1, p_t)
                s["w"] = w_t
            else:
                # h_out = (1-mu)*hb + mu*hhat
                nc.gpsimd.tensor_mul(p2, t1, s["hb"])
                s["w"] = rm[:, 2:4, :]
            s["p2"] = p2

        def back(g, t):
            """Step back half for group g: h matmuls, tanh, state update."""
            s = st[g]
            last = (t == T - 1)
            sps, rh = s["sps"], s["rh"]

            for k in range(2):
                for m in range(2):
                    nc.tensor.matmul(sps[:, 4 + m, :],
                                     wh_all[:, 8 + m * 2 + k, :],
                                     rh[:, k, :], start=False,
                                     stop=(m == 1 and k == 1))

            hhat = spool.tile([128, 2, GB], BF16, tag=f"hh{g}",
                              name=f"hh{g}")
            nc.scalar.activation(hhat, sps[:, 4:6, :], AF.Tanh)

            e_g = spool.tile([128, 2, GB], BF16, tag=f"e{g}", name=f"e{g}")
            nc.vector.tensor_mul(e_g, s["w"], hhat)
            if not last:
                nhb = spool.tile([128, 2, GB], BF16, tag=f"hb{g}",
                                 name=f"nhb{g}")
                nc.vector.tensor_add(nhb, s["p2"], e_g)
                hb_g[g] = nhb
            else:
                hout = spool.tile([128, 2, GB], FP32, tag=f"ho{g}",
                                  name=f"ho{g}")
                nc.vector.tensor_add(hout, s["p2"], e_g)
                nc.sync.dma_start(out_d[:, g * BL:(g + 1) * BL],
                                  hout.rearrange("p k b -> p (k b)"))

        gx_of[0] = cur_gx
        for c in range(NCHUNK):
            if c + 1 < NCHUNK:
                cur_x = xpool.tile([128, C * BL], BF16, tag="xch")
                nc.sync.dma_start(cur_x,
                                  xT[:, (c + 1) * C * BL:(c + 2) * C * BL])
                cur_gx = gxpool.tile([128, C, 2, 6, GB], BF16, tag="gx")
                gx_of[c + 1] = cur_gx

            for i in range(C):
                t = c * C + i
                # skewed pipeline: B runs half a step behind A
                front(0, t)
                if t > 0:
                    back(1, t - 1)
                front(1, t)
                back(0, t)

                # spread next chunk's precompute over this chunk's steps
                if c + 1 < NCHUNK and i in sched:
                    for u in sched[i]:
                        emit_pre_unit(c + 1, u)

        back(1, T - 1)

    nc.compile()
    return nc


def _prep_inputs(x, delta, W_mu, b_mu, W_r, b_r, W_h, b_h, W_td, b_td):
    bf = ml_dtypes.bfloat16
    # weights: first H rows act on h, last D rows act on x
    wh_gates = [W_r[:H], W_mu[:H], W_h[:H]]          # each [256, 256]
    wx_rmu = np.concatenate([W_r[H:], W_mu[H:]], axis=1)      # [128, 512]
    wx_h = W_h[H:]

    # contiguous [128,128] weight tiles, gate-major, k (contraction) minor
    tiles = []
    for gi, m in ((0, 0), (0, 1), (1, 0), (1, 1), (2, 0), (2, 1)):
        for k in range(2):
            tiles.append(wh_gates[gi][k * 128:(k + 1) * 128,
                                      m * 128:(m + 1) * 128])
    wh_all = np.concatenate(tiles, axis=1)                    # [128, 1536]

    def pcol(v):  # [2*128] -> [128, 2] column-per-tile
        return np.ascontiguousarray(np.stack([v[:128], v[128:]], axis=1),
                                    dtype=np.float32)

    b_rmu_col = np.concatenate([b_r, b_mu])                    # [512]
    b_rmu_t = np.ascontiguousarray(
        np.stack([b_rmu_col[i * 128:(i + 1) * 128] for i in range(4)], axis=1),
        dtype=np.float32)                                      # [128, 4]

    shared = {
        "wx_rmu": np.ascontiguousarray(wx_rmu, dtype=bf),
        "wx_h": np.ascontiguousarray(wx_h, dtype=bf),
        "wtd": np.ascontiguousarray(W_td, dtype=bf),
        "wh_all": np.ascontiguousarray(wh_all, dtype=bf),
        "b_rmu": b_rmu_t,
        "b_h": pcol(b_h),
        "nb_td": pcol(-b_td),
    }

    # delta shifted by one step: beta used at step t is beta(t+1)
    dshift = np.concatenate(
        [delta[:, 1:, :], np.zeros((B, 1, D), np.float32)], axis=1)

    in_maps = []
    for ci in range(NCORES):
        xs = x[ci * BL:(ci + 1) * BL]          # [32, 512, 128]
        ds = dshift[ci * BL:(ci + 1) * BL]
        # [BL, T, D] -> [D, T, BL] -> [D, T*BL]  (column t*BL + b)
        xt = np.ascontiguousarray(
            xs.transpose(2, 1, 0).reshape(D, T * BL), dtype=bf)
        dt_ = np.ascontiguousarray(
            ds.transpose(2, 1, 0).reshape(D, T * BL), dtype=bf)
        in_maps.append({"xT": xt, "dTs": dt_, **shared})
    return in_maps


def kernel(x, delta, W_mu, b_mu, W_r, b_r, W_h, b_h, W_td, b_td):
    args = tuple(np.asarray(a, dtype=np.float32) for a in
                 (x, delta, W_mu, b_mu, W_r, b_r, W_h, b_h, W_td, b_td))
    in_maps = _prep_inputs(*args)
    if "nc" not in _cache:
        _cache["nc"] = _build()
    res = run_bass_kernel_spmd(_cache["nc"], in_maps,
                               core_ids=list(range(NCORES)))
    out = np.empty((B, H), np.float32)
    for ci in range(NCORES):
        o = res.results[ci]["hT_out"]          # [128, 2*BL]; col = g*32+k*16+b
        for g in range(2):
            for k in range(2):
                out[ci * BL + g * GB:ci * BL + (g + 1) * GB,
                    k * 128:(k + 1) * 128] = \
                    o[:, g * 32 + k * GB:g * 32 + (k + 1) * GB].T
    return out
